# revision 15
# baseline (speedup 1.0000x reference)
"""Trainium2 Bass kernel for nn_ArrivalTime (8-core data-parallel), v4.

Math restructure (exact): with T=24 timeslots and one user per batch row,
scores for token n are row  D[b(n)*24 + hour(n)]  of a small [32, 96]
table D = [C; A]:
    C[tau, h*24+t] = SCALE * (ts_tau @ W_C)        (timeslot part of q) . k
    A[b,   h*24+t] = SCALE * (ur_b @ W_A + bqk)    (user part of q + bq) . k
where W_C/W_A/bqk fold the k-projection into the q-projection on the host
(token-independent weight preprocessing, 1024x96 each); the row select +
mask-add is a one-hot matmul  S_tile = ohm.T @ acm  with acm = [C; A; mrows].
After per-head softmax over t, out = attn_flat[n, 96] @ vu + implicit bu,
with vu[h*24+t, :] = v[h,t] @ Wu_h.T + bu/4 folded on the host as well.

Device pipeline (per core, 8 batch rows -> 4096 tokens, 32 tiles of 128):
  front:  16 small matmuls build acm[0:32] from device-side ts/user rows.
  loop over 8 groups of 4 tiles, software-pipelined one group ahead:
    PE:     4x score matmuls [56,128]x[56,96] -> psum
    Scalar: exp activation psum -> sbuf bf16
    GpSimd: row-sum reduce + reciprocal-multiply (softmax, SBUF-only)
    PE:     per-tile transpose attn -> [96,128], out GEMM [96,128]x[96,1024]
    V/S:    psum->sbuf bf16 evacuation, split po0/po1 per engine
    Sync:   output DMA per 4-tile chunk on the hardware DGE queue
Engine budget/tile ~680ns V/S balanced; PE warmed up at start to hold the
2.4 GHz p-state.  No collectives (measured ~80us for a 53KB AllGather).
"""

import numpy as np
import ml_dtypes
from contextlib import ExitStack

import concourse.bass as bass
import concourse.mybir as mybir
import concourse.tile as tile
from concourse import bacc
from concourse.masks import make_identity
from concourse.bass_utils import run_bass_kernel_spmd

F32 = mybir.dt.float32
BF16 = mybir.dt.bfloat16
AF = mybir.ActivationFunctionType
ALU = mybir.AluOpType

D_MODEL = 1024
N_HEADS = 4
HEAD_DIM = 256
T = 24
B, S = 64, 512
NCORES = 8
BL = B // NCORES            # 8 batch rows per core
NL = BL * S                 # 4096 tokens per core
P = 128
NT = NL // P                # 32 token tiles
HT = N_HEADS * T            # 96
GRP = 4                     # token tiles per softmax group
NG = NT // GRP              # 8 groups
KC = 8                      # contraction chunks of 128
SCALE = 1.0 / np.sqrt(HEAD_DIM)
NEG_BIG = np.float32(-1e30)
N_WARM = 8                  # 512-wide warm matmuls bridge the input DMAs
VSPLIT = 456                # V evacuates out cols [0:496], S takes the rest
NR = 64                     # acm/ohm rows: C 0:24 | pad | A 32:40 | mask 40:64

# engine assignment for the softmax reduce/mult (G = gpsimd, V = vector)
RED_ENG = "vector"   # gpsimd tensor_reduce can't do free-dim (X) reductions
MUL_ENG = "gpsimd"


def build():
    nc = bacc.Bacc("TRN2", target_bir_lowering=False, debug=False)

    xt = nc.dram_tensor("xt", [P, KC, 32], BF16, kind="ExternalInput")
    wca = nc.dram_tensor("wca", [P, KC, 2 * HT], BF16, kind="ExternalInput")
    bqk = nc.dram_tensor("bqk", [1, HT], BF16, kind="ExternalInput")
    mrows = nc.dram_tensor("mrows", [T, HT], BF16, kind="ExternalInput")
    ohm = nc.dram_tensor("ohm", [NR, NL], BF16, kind="ExternalInput")
    vut = nc.dram_tensor("vut", [HT, D_MODEL], BF16, kind="ExternalInput")
    outD = nc.dram_tensor("outD", [P, NT, D_MODEL], BF16,
                          kind="ExternalOutput")

    with tile.TileContext(nc) as tc, ExitStack() as ctx:
        const = ctx.enter_context(tc.tile_pool(name="const", bufs=1))
        sb = ctx.enter_context(tc.tile_pool(name="sb", bufs=2))
        obp = ctx.enter_context(tc.tile_pool(name="obp", bufs=3))

        identb = const.tile([P, P], BF16)
        make_identity(nc, identb[:])
        ones8 = const.tile([1, BL], BF16)
        nc.vector.memset(ones8[:], 1.0)
        warm_sb = const.tile([P, P], BF16)
        nc.vector.memset(warm_sb[:], 0.0)
        vu_pad = const.tile([P, 512], BF16)
        nc.vector.memset(vu_pad[:], 0.0)

        # front-critical small tensors on the gpsimd queue (parallel with
        # sync's stream); gpsimd is otherwise idle until the first softmax.
        xt_sb = const.tile([P, KC, 32], BF16)
        nc.gpsimd.dma_start(xt_sb[:], xt[:])
        acm = const.tile([NR, HT], BF16)
        # pad rows 24:32 are contracted against zero ohm rows, but 0*garbage
        # can be NaN — zero the whole table before filling it.
        nc.vector.memset(acm[:], 0.0)
        nc.gpsimd.dma_start(acm[40:40 + T, :], mrows[:])
        bqk_sb = const.tile([1, HT], BF16)
        nc.gpsimd.dma_start(bqk_sb[:], bqk[:])

        # sync hardware queue: score-table weights, then ohm chunks in
        # consumption order, then the value table.
        wca_sb = const.tile([P, KC, 2 * HT], BF16)
        nc.sync.dma_start(wca_sb[:], wca[:])
        ohm_sb = const.tile([NR, NL], BF16)
        for i in range(4):
            nc.sync.dma_start(ohm_sb[:, 1024 * i:1024 * (i + 1)],
                              ohm[:, 1024 * i:1024 * (i + 1)])
        vu_sb = const.tile([HT, D_MODEL], BF16)
        nc.sync.dma_start(vu_sb[:], vut[:])

        with tc.tile_pool(name="ps", bufs=2, space="PSUM") as psp:
            # PE warm-up: bf16 matmuls keep the HAM busy-window alive while
            # the first DMAs land, ramping the PE clock toward 2.4 GHz.
            warm_ps = psp.tile([P, 1024], F32, tag="po", bufs=3, name="warm")
            for i in range(N_WARM):
                for j in range(2):
                    nc.tensor.matmul(warm_ps[:, 512 * j:512 * (j + 1)],
                                     warm_sb[:], vu_pad[:],
                                     start=(i == 0), stop=(i == N_WARM - 1))
            warm_out = const.tile([P, 8], F32)
            nc.vector.tensor_copy(warm_out[:], warm_ps[:, 0:8])

            def emit_bridge(n, tag_name):
                # dense zero-matmuls: keep the PE busy-window alive across a
                # dependency gap so the HAM doesn't re-throttle the clock.
                br = psp.tile([P, P], F32, tag="sc", bufs=1, name=tag_name)
                for i in range(n):
                    nc.tensor.matmul(br[:], warm_sb[:], warm_sb[:],
                                     start=(i == 0), stop=(i == n - 1))
                nc.vector.tensor_copy(warm_out[:], br[:, 0:8])

            # ---- front-end: acm[0:24] = C = ts @ W_C ; acm[24:32] = A ----
            psac = psp.tile([40, HT], F32, tag="sc", bufs=1, name="psac")
            for c in range(KC):
                nc.tensor.matmul(psac[0:T, :], xt_sb[:, c, 0:T],
                                 wca_sb[:, c, 0:HT],
                                 start=(c == 0), stop=(c == KC - 1),
                                 tile_position=(0, 0), skip_group_check=True)
            for c in range(KC):
                nc.tensor.matmul(psac[32:40, :], xt_sb[:, c, T:32],
                                 wca_sb[:, c, HT:2 * HT],
                                 start=(c == 0), stop=False,
                                 tile_position=(0, 32), skip_group_check=True)
            nc.tensor.matmul(psac[32:40, :], ones8[:], bqk_sb[:],
                             start=False, stop=True,
                             tile_position=(0, 32), skip_group_check=True)
            nc.vector.tensor_copy(acm[0:T, :], psac[0:T, :])
            nc.vector.tensor_copy(acm[32:40, :], psac[32:40, :])

            # ---- token pipeline ----
            sge_t, att_t = {}, {}

            def emit_scores(g):
                # gp = g // 2: sge tiles span two 4-tile score groups so the
                # softmax vector ops amortize their fixed cost over 8 tiles.
                psc = psp.tile([P, GRP, HT], F32, tag="sc", bufs=1,
                               name=f"psc{g}")
                for u in range(GRP):
                    a = g * GRP + u
                    nc.tensor.matmul(psc[:, u, :], ohm_sb[:, a * P:(a + 1) * P],
                                     acm[:], start=True, stop=True)
                if g % 2 == 0:
                    sge_t[g // 2] = sb.tile([P, 2, GRP, HT], BF16, tag="sge",
                                            bufs=2, name=f"sge{g // 2}")
                sge = sge_t[g // 2]
                nc.scalar.activation(sge[:, g % 2], psc[:], AF.Exp)

            def emit_softmax(gp):
                sge = sge_t.pop(gp)
                scv = sge[:].rearrange("p a g (h t) -> p a g h t", h=N_HEADS)
                hs = sb.tile([P, 2, GRP, N_HEADS], F32, tag="hs", bufs=2,
                             name=f"hs{gp}")
                nc.vector.reduce_sum(hs[:], scv, axis=mybir.AxisListType.X)
                nc.vector.reciprocal(hs[:], hs[:])
                att = sb.tile([P, 2, GRP, HT], BF16, tag="att", bufs=2,
                              name=f"att{gp}")
                attv = att[:].rearrange("p a g (h t) -> p a g h t", h=N_HEADS)
                rb = hs[:, :, :, :, None].broadcast_to(
                    [P, 2, GRP, N_HEADS, T])
                mul = getattr(nc, MUL_ENG)
                mul.tensor_tensor(out=attv, in0=scv, in1=rb, op=ALU.mult)
                att_t[gp] = att

            def emit_tokens(g):
                att = att_t[g // 2]
                ob = obp.tile([P, GRP, D_MODEL], BF16, tag="ob", bufs=3,
                              name=f"ob{g}")
                for u in range(GRP):
                    t = g * GRP + u
                    tp = psp.tile([HT, P], BF16, tag="tp", bufs=1,
                                  name=f"tp{t}")
                    nc.tensor.transpose(tp[:], att[:, g % 2, u, :], identb[:])
                    atT = sb.tile([HT, P], BF16, tag="atT", bufs=4,
                                  name=f"atT{t}")
                    if u % 4 != 3:
                        nc.vector.tensor_copy(atT[:], tp[:])
                    else:
                        nc.scalar.copy(atT[:], tp[:])
                    po = psp.tile([P, 1024], F32, tag="po", bufs=3,
                                  name=f"po{t}")
                    for j in range(2):
                        nc.tensor.matmul(po[:, 512 * j:512 * (j + 1)],
                                         atT[:], vu_sb[:, 512 * j:512 * (j + 1)],
                                         start=True, stop=True)
                    nc.vector.tensor_copy(ob[:, u, 0:VSPLIT], po[:, 0:VSPLIT])
                    nc.scalar.copy(ob[:, u, VSPLIT:], po[:, VSPLIT:])
                dst = outD[:, g * GRP:(g + 1) * GRP, :]
                if g % 2 == 0:
                    nc.sync.dma_start(dst, ob[:])
                else:
                    nc.gpsimd.dma_start(dst, ob[:])

            # software pipeline, one 8-tile super-group ahead: scores a/b ->
            # softmax8 -> tokens, with a PE keepalive bridge across the first
            # softmax latency gap.
            emit_scores(0)
            emit_scores(1)
            emit_softmax(0)
            emit_bridge(40, "bridge0")
            for gp in range(1, NG // 2):
                emit_scores(2 * gp)
                emit_scores(2 * gp + 1)
                emit_softmax(gp)
                emit_tokens(2 * gp - 2)
                emit_tokens(2 * gp - 1)
            emit_tokens(NG - 2)
            emit_tokens(NG - 1)

    nc.finalize()
    return nc


def _bf16(x):
    return np.ascontiguousarray(np.asarray(x).astype(ml_dtypes.bfloat16))


def _pmajor(x):
    """[KC, 128, X] -> [128, KC, X] partition-major host layout."""
    return np.ascontiguousarray(np.transpose(x, (1, 0, 2)))


def prep_in_maps(inputs):
    ts = np.asarray(inputs["timeslot_embedded"], np.float32)
    user_x1 = np.asarray(inputs["user_x1"]).astype(np.int64)
    hour = np.asarray(inputs["hour_x1"]).astype(np.int64)
    mask = np.asarray(inputs["hour_mask1"]).astype(np.int64)
    up = np.asarray(inputs["up_table"], np.float32)
    Wq = np.asarray(inputs["Wq"], np.float32).reshape(D_MODEL, 2 * D_MODEL)
    bq = np.asarray(inputs["bq"], np.float32).ravel()
    Wk = np.asarray(inputs["Wk"], np.float32).reshape(D_MODEL, D_MODEL)
    bk = np.asarray(inputs["bk"], np.float32).ravel()
    Wv = np.asarray(inputs["Wv"], np.float32).reshape(D_MODEL, D_MODEL)
    bv = np.asarray(inputs["bv"], np.float32).ravel()
    Wu = np.asarray(inputs["Wu"], np.float32)
    bu = np.asarray(inputs["bu"], np.float32)

    Wq_u, Wq_t = Wq[:, :D_MODEL], Wq[:, D_MODEL:]

    # token-independent weight folding (host): fold k into the q-side score
    # tables, and Wu/bu into the value table.
    k = ts @ Wk.T + bk                       # [24, 1024], head-blocked cols
    v = ts @ Wv.T + bv
    W_C = np.zeros((D_MODEL, HT), np.float32)
    W_A = np.zeros((D_MODEL, HT), np.float32)
    bqk_v = np.zeros(HT, np.float32)
    vu = np.zeros((HT, D_MODEL), np.float32)
    for h in range(N_HEADS):
        sl = slice(HEAD_DIM * h, HEAD_DIM * (h + 1))
        cs = slice(T * h, T * (h + 1))
        W_C[:, cs] = SCALE * (Wq_t[sl, :].T @ k[:, sl].T)
        W_A[:, cs] = SCALE * (Wq_u[sl, :].T @ k[:, sl].T)
        bqk_v[cs] = SCALE * (k[:, sl] @ bq[sl])
        vu[cs] = v[:, sl] @ Wu[:, sl].T
    vu += bu[None, :] / N_HEADS

    wca_m = _pmajor(_bf16(np.concatenate([W_C, W_A], axis=1)
                          .reshape(KC, P, 2 * HT)))
    bqk_m = _bf16(bqk_v[None, :])
    vu_m = _bf16(vu)

    # mrows[t', h*24+t] = -1e30 if t == t' else 0
    mr = np.zeros((T, HT), np.float32)
    for h in range(N_HEADS):
        mr[np.arange(T), h * T + np.arange(T)] = NEG_BIG
    mr = _bf16(mr)

    user_rows = up[user_x1].copy()
    user_rows[user_x1 == 0] = 0.0

    tok_b = (np.arange(NL) // S).astype(np.int64)
    in_maps = []
    for c in range(NCORES):
        ur = user_rows[c * BL:(c + 1) * BL]
        xt_m = _pmajor(_bf16(np.concatenate([ts.T, ur.T], axis=1)
                             .reshape(KC, P, 32)))
        hour_c = hour[c * BL:(c + 1) * BL].reshape(-1)
        ohe = np.zeros((NR, NL), np.float32)
        ohe[hour_c, np.arange(NL)] = 1.0          # tau selector -> C rows
        ohe[32 + tok_b, np.arange(NL)] = 1.0      # b selector -> A rows
        maskc = mask[c * BL:(c + 1) * BL].reshape(NL, T).astype(np.float32)
        ohe[40:40 + T, :] = maskc.T               # mask rows -> mrows
        in_maps.append({
            "xt": xt_m, "wca": wca_m, "bqk": bqk_m, "mrows": mr,
            "ohm": _bf16(ohe), "vut": vu_m,
        })
    return in_maps


_NC_CACHE = None


def get_nc():
    global _NC_CACHE
    if _NC_CACHE is None:
        _NC_CACHE = build()
    return _NC_CACHE


def run(inputs, trace=False, **kwargs):
    nc = get_nc()
    in_maps = prep_in_maps(inputs)
    res = run_bass_kernel_spmd(nc, in_maps, core_ids=list(range(NCORES)),
                               trace=trace, **kwargs)
    outs = []
    for r in res.results:
        full = np.asarray(r["outD"])
        outs.append(full.transpose(1, 0, 2).reshape(NL, D_MODEL))
    full = np.concatenate(outs, 0).reshape(B, S, D_MODEL).astype(np.float32)
    return full, res


def kernel(**inputs):
    full, _ = run(inputs, trace=False)
    return full


# revision 16
# speedup vs baseline: 1.2964x; 1.2964x over previous
"""Trainium2 Bass kernel for nn_ArrivalTime (8-core data-parallel), v4.

Math restructure (exact): with T=24 timeslots and one user per batch row,
scores for token n are row  D[b(n)*24 + hour(n)]  of a small [32, 96]
table D = [C; A]:
    C[tau, h*24+t] = SCALE * (ts_tau @ W_C)        (timeslot part of q) . k
    A[b,   h*24+t] = SCALE * (ur_b @ W_A + bqk)    (user part of q + bq) . k
where W_C/W_A/bqk fold the k-projection into the q-projection on the host
(token-independent weight preprocessing, 1024x96 each); the row select +
mask-add is a one-hot matmul  S_tile = ohm.T @ acm  with acm = [C; A; mrows].
After per-head softmax over t, out = attn_flat[n, 96] @ vu + implicit bu,
with vu[h*24+t, :] = v[h,t] @ Wu_h.T + bu/4 folded on the host as well.

Device pipeline (per core, 8 batch rows -> 4096 tokens, 32 tiles of 128):
  front:  16 small matmuls build acm[0:32] from device-side ts/user rows.
  loop over 8 groups of 4 tiles, software-pipelined one group ahead:
    PE:     4x score matmuls [56,128]x[56,96] -> psum
    Scalar: exp activation psum -> sbuf bf16
    GpSimd: row-sum reduce + reciprocal-multiply (softmax, SBUF-only)
    PE:     per-tile transpose attn -> [96,128], out GEMM [96,128]x[96,1024]
    V/S:    psum->sbuf bf16 evacuation, split po0/po1 per engine
    Sync:   output DMA per 4-tile chunk on the hardware DGE queue
Engine budget/tile ~680ns V/S balanced; PE warmed up at start to hold the
2.4 GHz p-state.  No collectives (measured ~80us for a 53KB AllGather).
"""

import numpy as np
import ml_dtypes
from contextlib import ExitStack

import concourse.bass as bass
import concourse.mybir as mybir
import concourse.tile as tile
from concourse import bacc
from concourse.masks import make_identity
from concourse.bass_utils import run_bass_kernel_spmd

F32 = mybir.dt.float32
BF16 = mybir.dt.bfloat16
AF = mybir.ActivationFunctionType
ALU = mybir.AluOpType

D_MODEL = 1024
N_HEADS = 4
HEAD_DIM = 256
T = 24
B, S = 64, 512
NCORES = 8
BL = B // NCORES            # 8 batch rows per core
NL = BL * S                 # 4096 tokens per core
P = 128
NT = NL // P                # 32 token tiles
HT = N_HEADS * T            # 96
GRP = 4                     # token tiles per softmax group
NG = NT // GRP              # 8 groups
KC = 8                      # contraction chunks of 128
SCALE = 1.0 / np.sqrt(HEAD_DIM)
NEG_BIG = np.float32(-1e30)
N_WARM = 8                  # 512-wide warm matmuls bridge the input DMAs
VSPLIT = 456                # V evacuates out cols [0:496], S takes the rest
NR = 64                     # acm/ohm rows: C 0:24 | pad | A 32:40 | mask 40:64

# engine assignment for the softmax reduce/mult (G = gpsimd, V = vector)
RED_ENG = "vector"   # gpsimd tensor_reduce can't do free-dim (X) reductions
MUL_ENG = "gpsimd"


def build():
    nc = bacc.Bacc("TRN2", target_bir_lowering=False, debug=False)

    xt = nc.dram_tensor("xt", [P, KC, 32], BF16, kind="ExternalInput")
    wca = nc.dram_tensor("wca", [P, KC, 2 * HT], BF16, kind="ExternalInput")
    bqk = nc.dram_tensor("bqk", [1, HT], BF16, kind="ExternalInput")
    mrows = nc.dram_tensor("mrows", [T, HT], BF16, kind="ExternalInput")
    ohm = nc.dram_tensor("ohm", [NR, NL], BF16, kind="ExternalInput")
    vut = nc.dram_tensor("vut", [HT, D_MODEL], BF16, kind="ExternalInput")
    outD = nc.dram_tensor("outD", [P, NT, D_MODEL], BF16,
                          kind="ExternalOutput")

    with tile.TileContext(nc) as tc, ExitStack() as ctx:
        const = ctx.enter_context(tc.tile_pool(name="const", bufs=1))
        sb = ctx.enter_context(tc.tile_pool(name="sb", bufs=2))
        obp = ctx.enter_context(tc.tile_pool(name="obp", bufs=3))

        identb = const.tile([P, P], BF16)
        make_identity(nc, identb[:])
        ones8 = const.tile([1, BL], BF16)
        nc.vector.memset(ones8[:], 1.0)
        warm_sb = const.tile([P, P], BF16)
        nc.vector.memset(warm_sb[:], 0.0)
        vu_pad = const.tile([P, 512], BF16)
        nc.vector.memset(vu_pad[:], 0.0)

        # front-critical small tensors on the gpsimd queue (parallel with
        # sync's stream); gpsimd is otherwise idle until the first softmax.
        xt_sb = const.tile([P, KC, 32], BF16)
        nc.gpsimd.dma_start(xt_sb[:], xt[:])
        acm = const.tile([NR, HT], BF16)
        # pad rows 24:32 are contracted against zero ohm rows, but 0*garbage
        # can be NaN — zero the whole table before filling it.
        nc.vector.memset(acm[:], 0.0)
        nc.gpsimd.dma_start(acm[40:40 + T, :], mrows[:])
        bqk_sb = const.tile([1, HT], BF16)
        nc.gpsimd.dma_start(bqk_sb[:], bqk[:])

        # sync hardware queue: score-table weights, then ohm chunks in
        # consumption order, then the value table.
        wca_sb = const.tile([P, KC, 2 * HT], BF16)
        nc.sync.dma_start(wca_sb[:], wca[:])
        ohm_sb = const.tile([NR, NL], BF16)
        for i in range(4):
            nc.sync.dma_start(ohm_sb[:, 1024 * i:1024 * (i + 1)],
                              ohm[:, 1024 * i:1024 * (i + 1)])
        vu_sb = const.tile([HT, D_MODEL], BF16)
        nc.sync.dma_start(vu_sb[:], vut[:])

        with tc.tile_pool(name="ps", bufs=2, space="PSUM") as psp:
            # PE warm-up: bf16 matmuls keep the HAM busy-window alive while
            # the first DMAs land, ramping the PE clock toward 2.4 GHz.
            warm_ps = psp.tile([P, 512], F32, tag="po", bufs=4, name="warm")
            for i in range(2 * N_WARM):
                nc.tensor.matmul(warm_ps[:], warm_sb[:], vu_pad[:],
                                 start=(i == 0), stop=(i == 2 * N_WARM - 1))
            warm_out = const.tile([P, 8], F32)
            nc.vector.tensor_copy(warm_out[:], warm_ps[:, 0:8])

            def emit_bridge(n, tag_name):
                # dense zero-matmuls: keep the PE busy-window alive across a
                # dependency gap so the HAM doesn't re-throttle the clock.
                br = psp.tile([P, P], F32, tag="sc", bufs=2, name=tag_name)
                for i in range(n):
                    nc.tensor.matmul(br[:], warm_sb[:], warm_sb[:],
                                     start=(i == 0), stop=(i == n - 1))
                nc.vector.tensor_copy(warm_out[:], br[:, 0:8])

            # ---- front-end: acm[0:24] = C = ts @ W_C ; acm[24:32] = A ----
            psac = psp.tile([40, HT], F32, tag="sc", bufs=2, name="psac")
            for c in range(KC):
                nc.tensor.matmul(psac[0:T, :], xt_sb[:, c, 0:T],
                                 wca_sb[:, c, 0:HT],
                                 start=(c == 0), stop=(c == KC - 1),
                                 tile_position=(0, 0), skip_group_check=True)
            for c in range(KC):
                nc.tensor.matmul(psac[32:40, :], xt_sb[:, c, T:32],
                                 wca_sb[:, c, HT:2 * HT],
                                 start=(c == 0), stop=False,
                                 tile_position=(0, 32), skip_group_check=True)
            nc.tensor.matmul(psac[32:40, :], ones8[:], bqk_sb[:],
                             start=False, stop=True,
                             tile_position=(0, 32), skip_group_check=True)
            nc.vector.tensor_copy(acm[0:T, :], psac[0:T, :])
            nc.vector.tensor_copy(acm[32:40, :], psac[32:40, :])

            # ---- token pipeline ----
            sge_t, att_t = {}, {}

            def emit_scores(g):
                # gp = g // 2: sge tiles span two 4-tile score groups so the
                # softmax vector ops amortize their fixed cost over 8 tiles.
                psc = psp.tile([P, GRP, HT], F32, tag="sc", bufs=2,
                               name=f"psc{g}")
                for u in range(GRP):
                    a = g * GRP + u
                    nc.tensor.matmul(psc[:, u, :], ohm_sb[:, a * P:(a + 1) * P],
                                     acm[:], start=True, stop=True)
                if g % 2 == 0:
                    sge_t[g // 2] = sb.tile([P, 2, GRP, HT], BF16, tag="sge",
                                            bufs=2, name=f"sge{g // 2}")
                sge = sge_t[g // 2]
                nc.scalar.activation(sge[:, g % 2], psc[:], AF.Exp)

            def emit_softmax(gp):
                sge = sge_t.pop(gp)
                scv = sge[:].rearrange("p a g (h t) -> p a g h t", h=N_HEADS)
                hs = sb.tile([P, 2, GRP, N_HEADS], F32, tag="hs", bufs=2,
                             name=f"hs{gp}")
                nc.vector.reduce_sum(hs[:], scv, axis=mybir.AxisListType.X)
                nc.vector.reciprocal(hs[:], hs[:])
                att = sb.tile([P, 2, GRP, HT], BF16, tag="att", bufs=2,
                              name=f"att{gp}")
                attv = att[:].rearrange("p a g (h t) -> p a g h t", h=N_HEADS)
                rb = hs[:, :, :, :, None].broadcast_to(
                    [P, 2, GRP, N_HEADS, T])
                mul = getattr(nc, MUL_ENG)
                mul.tensor_tensor(out=attv, in0=scv, in1=rb, op=ALU.mult)
                att_t[gp] = att

            def emit_tokens(g):
                att = att_t[g // 2]
                ob = obp.tile([P, GRP, D_MODEL], BF16, tag="ob", bufs=3,
                              name=f"ob{g}")
                for u in range(GRP):
                    t = g * GRP + u
                    tp = psp.tile([HT, P], BF16, tag="tp", bufs=2,
                                  name=f"tp{t}")
                    nc.tensor.transpose(tp[:], att[:, g % 2, u, :], identb[:])
                    atT = sb.tile([HT, P], BF16, tag="atT", bufs=4,
                                  name=f"atT{t}")
                    if u % 2 == 0:
                        nc.vector.tensor_copy(atT[:], tp[:])
                    else:
                        nc.scalar.copy(atT[:], tp[:])
                    po0 = psp.tile([P, 512], F32, tag="po", bufs=4,
                                   name=f"po0_{t}")
                    nc.tensor.matmul(po0[:], atT[:], vu_sb[:, 0:512],
                                     start=True, stop=True)
                    po1 = psp.tile([P, 512], F32, tag="po", bufs=4,
                                   name=f"po1_{t}")
                    nc.tensor.matmul(po1[:], atT[:], vu_sb[:, 512:1024],
                                     start=True, stop=True)
                    nc.vector.tensor_copy(ob[:, u, 0:512], po0[:])
                    nc.scalar.copy(ob[:, u, 512:], po1[:])
                dst = outD[:, g * GRP:(g + 1) * GRP, :]
                if g % 2 == 0:
                    nc.sync.dma_start(dst, ob[:])
                else:
                    nc.gpsimd.dma_start(dst, ob[:])

            # software pipeline, one 8-tile super-group ahead: scores a/b ->
            # softmax8 -> tokens, with a PE keepalive bridge across the first
            # softmax latency gap.
            emit_scores(0)
            emit_scores(1)
            emit_softmax(0)
            emit_bridge(40, "bridge0")
            for gp in range(1, NG // 2):
                emit_scores(2 * gp)
                emit_scores(2 * gp + 1)
                emit_softmax(gp)
                emit_tokens(2 * gp - 2)
                emit_tokens(2 * gp - 1)
            emit_tokens(NG - 2)
            emit_tokens(NG - 1)

    nc.finalize()
    return nc


def _bf16(x):
    return np.ascontiguousarray(np.asarray(x).astype(ml_dtypes.bfloat16))


def _pmajor(x):
    """[KC, 128, X] -> [128, KC, X] partition-major host layout."""
    return np.ascontiguousarray(np.transpose(x, (1, 0, 2)))


def prep_in_maps(inputs):
    ts = np.asarray(inputs["timeslot_embedded"], np.float32)
    user_x1 = np.asarray(inputs["user_x1"]).astype(np.int64)
    hour = np.asarray(inputs["hour_x1"]).astype(np.int64)
    mask = np.asarray(inputs["hour_mask1"]).astype(np.int64)
    up = np.asarray(inputs["up_table"], np.float32)
    Wq = np.asarray(inputs["Wq"], np.float32).reshape(D_MODEL, 2 * D_MODEL)
    bq = np.asarray(inputs["bq"], np.float32).ravel()
    Wk = np.asarray(inputs["Wk"], np.float32).reshape(D_MODEL, D_MODEL)
    bk = np.asarray(inputs["bk"], np.float32).ravel()
    Wv = np.asarray(inputs["Wv"], np.float32).reshape(D_MODEL, D_MODEL)
    bv = np.asarray(inputs["bv"], np.float32).ravel()
    Wu = np.asarray(inputs["Wu"], np.float32)
    bu = np.asarray(inputs["bu"], np.float32)

    Wq_u, Wq_t = Wq[:, :D_MODEL], Wq[:, D_MODEL:]

    # token-independent weight folding (host): fold k into the q-side score
    # tables, and Wu/bu into the value table.
    k = ts @ Wk.T + bk                       # [24, 1024], head-blocked cols
    v = ts @ Wv.T + bv
    W_C = np.zeros((D_MODEL, HT), np.float32)
    W_A = np.zeros((D_MODEL, HT), np.float32)
    bqk_v = np.zeros(HT, np.float32)
    vu = np.zeros((HT, D_MODEL), np.float32)
    for h in range(N_HEADS):
        sl = slice(HEAD_DIM * h, HEAD_DIM * (h + 1))
        cs = slice(T * h, T * (h + 1))
        W_C[:, cs] = SCALE * (Wq_t[sl, :].T @ k[:, sl].T)
        W_A[:, cs] = SCALE * (Wq_u[sl, :].T @ k[:, sl].T)
        bqk_v[cs] = SCALE * (k[:, sl] @ bq[sl])
        vu[cs] = v[:, sl] @ Wu[:, sl].T
    vu += bu[None, :] / N_HEADS

    wca_m = _pmajor(_bf16(np.concatenate([W_C, W_A], axis=1)
                          .reshape(KC, P, 2 * HT)))
    bqk_m = _bf16(bqk_v[None, :])
    vu_m = _bf16(vu)

    # mrows[t', h*24+t] = -1e30 if t == t' else 0
    mr = np.zeros((T, HT), np.float32)
    for h in range(N_HEADS):
        mr[np.arange(T), h * T + np.arange(T)] = NEG_BIG
    mr = _bf16(mr)

    user_rows = up[user_x1].copy()
    user_rows[user_x1 == 0] = 0.0

    tok_b = (np.arange(NL) // S).astype(np.int64)
    in_maps = []
    for c in range(NCORES):
        ur = user_rows[c * BL:(c + 1) * BL]
        xt_m = _pmajor(_bf16(np.concatenate([ts.T, ur.T], axis=1)
                             .reshape(KC, P, 32)))
        hour_c = hour[c * BL:(c + 1) * BL].reshape(-1)
        ohe = np.zeros((NR, NL), np.float32)
        ohe[hour_c, np.arange(NL)] = 1.0          # tau selector -> C rows
        ohe[32 + tok_b, np.arange(NL)] = 1.0      # b selector -> A rows
        maskc = mask[c * BL:(c + 1) * BL].reshape(NL, T).astype(np.float32)
        ohe[40:40 + T, :] = maskc.T               # mask rows -> mrows
        in_maps.append({
            "xt": xt_m, "wca": wca_m, "bqk": bqk_m, "mrows": mr,
            "ohm": _bf16(ohe), "vut": vu_m,
        })
    return in_maps


_NC_CACHE = None


def get_nc():
    global _NC_CACHE
    if _NC_CACHE is None:
        _NC_CACHE = build()
    return _NC_CACHE


def run(inputs, trace=False, **kwargs):
    nc = get_nc()
    in_maps = prep_in_maps(inputs)
    res = run_bass_kernel_spmd(nc, in_maps, core_ids=list(range(NCORES)),
                               trace=trace, **kwargs)
    outs = []
    for r in res.results:
        full = np.asarray(r["outD"])
        outs.append(full.transpose(1, 0, 2).reshape(NL, D_MODEL))
    full = np.concatenate(outs, 0).reshape(B, S, D_MODEL).astype(np.float32)
    return full, res


def kernel(**inputs):
    full, _ = run(inputs, trace=False)
    return full
